# revision 20
# baseline (speedup 1.0000x reference)
"""Shapelet distance transform kernel for Trainium2 (8 NeuronCores).

out[b, s] = min_w sum_{l,c} (data[b, w+l, c] - kernel[s, l, c])^2 / LS

Strategy (data-parallel over batch, 4 batches per core, no collectives):
  dist[s, w] = (a2[w] + k2[s] - 2*cross[s, w]) / LS

Per core, per batch:
  - d2[t] = sum_c data[t, c]^2 staged to DRAM scratch (one pass).
  - Stride-16 window loads: T[p, j] = flat[3*(w0 + 16p) + j], shape [128, 141],
    contains the im2col rows of 2048 windows as column slices T[:, 3q:3q+96]
    (window w0 + 16p + q lives in partition p, slice q). One 73 KB DMA with
    564-byte contiguous lines covers 2048 windows - the min over windows is
    order-agnostic so the odd window order is free.
  - D[p, j] = d2[w0 + 16p + j], shape [128, 47]; slice D[:, q:q+32] holds the
    32 sliding-norm taps of the same windows.
  - ACT assembles per-q [x(96) | d2(32)] column blocks (PE matmul dst
    must start at partition 0, so one combined transpose per q is used);
    ACT copies 4 transposed blocks to SBUF as the matmul rhs [128, 512].
  - One float32r matmul per (s-chunk, rhs tile): lhsT = Kaug[128, 128] with
    rows [0:96] = -2*kernel^T/LS and rows [96:128] = 1/LS (turning the taps
    into a2[w]/LS), so PSUM = (a2 - 2*cross)/LS directly.
  - min over w: DVE tensor_reduce(min) straight from PSUM (the DVE's single
    PSUM read port makes 1 elem/lane/cycle the hard floor; offload paths all
    measured slower).
  - + k2[s]/LS, then DMA out.
"""

import sys

for _p in ("/opt/trn_rl_repo",):
    if _p not in sys.path:
        sys.path.insert(0, _p)

from contextlib import ExitStack

import ml_dtypes
import numpy as np

import concourse.bacc as bacc
import concourse.bass as bass
import concourse.tile as tile
from concourse import mybir

F32 = mybir.dt.float32
F32R = mybir.dt.float32r
BF16 = mybir.dt.bfloat16

# X-side dtype: bfloat16 halves PE stream/weight-load time at ~1e-3 rel err;
# float32r gives ~1.7e-4 rel err at ~2x the PE cost.
USE_BF16 = True
XDT = BF16 if USE_BF16 else F32R

B, T, C = 32, 8192, 3
NS, LS = 256, 32
W = T - LS + 1  # 8161 valid windows
NCORES = 8
BL = B // NCORES  # 4 batches per core
FLAT = T * C  # 24576
KC = LS * C  # 96 im2col columns from data
SCALE = 1.0 / LS

# Each T-load covers 2048 windows: w = w0 + 16p + q, p in [0,128), q in [0,16).
# Final load overlaps so that [0, W) is covered exactly in-bounds.
TLOADS = [0, 2048, 4096, W - 2048]
TCOLS = 3 * 15 + KC  # 141
DCOLS = 15 + LS  # 47
# Per T-load: 4 rhs tiles (q-groups of 4), paired into 2 PSUM min-groups.
# Per (batch, s-chunk): 8 min-groups. Groups routed via ACT copy + 2x DVE min:
ROUTED = set()  # HW: routed min costs MORE DVE time than direct PSUM reduce


def build_program() -> bass.Bass:
    nc = bacc.Bacc("TRN2", target_bir_lowering=False, debug=False)
    data = nc.dram_tensor("data", [BL, FLAT], XDT, kind="ExternalInput").ap()
    # pre[:, 0:96]=kern rows 0:128, [:, 96:192]=kern rows 128:256,
    # [:, 192:320]=identity(128); preb = bf16 identity (streams 2x faster)
    pre = nc.dram_tensor("pre", [128, 320], F32R, kind="ExternalInput").ap()
    preb = (
        nc.dram_tensor("preb", [128, 128], BF16, kind="ExternalInput").ap()
        if USE_BF16
        else None
    )
    out = nc.dram_tensor("out", [BL, NS], F32, kind="ExternalOutput").ap()
    d2s = nc.dram_tensor("d2s", [BL, T], XDT).ap()

    with tile.TileContext(nc) as tc, ExitStack() as ctx:
        consts = ctx.enter_context(tc.tile_pool(name="consts", bufs=1))
        pre_sb = consts.tile([128, 320], F32R)
        nc.sync.dma_start(out=pre_sb, in_=pre)
        ident = pre_sb[:, 192:320]
        if USE_BF16:
            identb = consts.tile([128, 128], BF16)
            nc.sync.dma_start(out=identb, in_=preb)
            identx = identb
        else:
            identx = ident
        kaug0 = consts.tile([128, 128], XDT)
        kaug1 = consts.tile([128, 128], XDT)
        kaug = [kaug0, kaug1]
        k2c0 = consts.tile([128, 1], F32)
        k2c1 = consts.tile([128, 1], F32)
        k2c = [k2c0, k2c1]

        # Phase-A pools stay alive for the whole kernel: releasing them would
        # add PSUM/SBUF reuse dependencies (and walrus-visible waits) onto the
        # first main-loop instructions. prep_ps keeps one dedicated PSUM bank
        # that later doubles as the dummy-transpose scratch.
        prep = ctx.enter_context(tc.tile_pool(name="prep", bufs=2))
        prep_ps = ctx.enter_context(tc.tile_pool(name="prep_ps", bufs=1, space="PSUM"))

        # ---- Phase A: stage d2, build Kaug and k2/LS columns (Tile
        # schedules by dependencies; program order here is not critical). ----
        for b in range(BL):
            dt_ = prep.tile([128, 192], XDT, tag="dt")
            nc.scalar.dma_start(out=dt_, in_=data[b].rearrange("(p f) -> p f", p=128))
            sq = prep.tile([128, 192], F32, tag="sq")
            nc.scalar.square(sq, dt_)
            d2t = prep.tile([128, 64], XDT, tag="d2t")
            # float32r is 4-byte fp32 storage; the accumulate is fp32.
            with nc.allow_low_precision(reason="f32r is fp32-width"):
                nc.vector.tensor_reduce(
                    d2t,
                    sq.rearrange("p (t c) -> p t c", c=3),
                    axis=mybir.AxisListType.X,
                    op=mybir.AluOpType.add,
                )
            nc.scalar.dma_start(out=d2s[b].rearrange("(p f) -> p f", p=128), in_=d2t)

        for sc in range(2):
            ksb = pre_sb[:, sc * KC : (sc + 1) * KC]
            ksq = prep.tile([128, KC], F32, tag="ksq")
            nc.vector.tensor_mul(ksq, ksb, ksb)
            k2raw = prep.tile([128, 1], F32, tag="k2raw")
            nc.vector.tensor_reduce(
                k2raw, ksq, axis=mybir.AxisListType.X, op=mybir.AluOpType.add
            )
            nc.vector.tensor_scalar_mul(k2c[sc], k2raw, SCALE)
            ktp = prep_ps.tile([KC, 128], F32R, tag="scratch")
            nc.tensor.transpose(ktp, ksb, ident)
            # rows 0:96 = -2/LS * K^T ; rows 96:128 = 1/LS (a2 taps)
            nc.scalar.activation(
                kaug[sc][0:KC, :],
                ktp,
                mybir.ActivationFunctionType.Copy,
                scale=-2.0 * SCALE,
            )
            # (memset can't target float32r; write the constant via DVE)
            nc.vector.tensor_scalar(
                out=kaug[sc][KC:128, :],
                in0=pre_sb[0:LS, 0:128],
                scalar1=0.0,
                scalar2=SCALE,
                op0=mybir.AluOpType.mult,
                op1=mybir.AluOpType.add,
            )

        # ---- Phase B: main loop. ----
        t_pool = ctx.enter_context(tc.tile_pool(name="tp", bufs=3))
        xq_pool = ctx.enter_context(tc.tile_pool(name="xq", bufs=4))
        d_pool = ctx.enter_context(tc.tile_pool(name="dp", bufs=3))
        xts_pool = ctx.enter_context(tc.tile_pool(name="xts", bufs=6))
        mins_pool = ctx.enter_context(tc.tile_pool(name="mins", bufs=4))
        fin_pool = ctx.enter_context(tc.tile_pool(name="fin", bufs=8))
        tr_ps = ctx.enter_context(tc.tile_pool(name="tr_ps", bufs=3, space="PSUM"))
        dist_ps = ctx.enter_context(tc.tile_pool(name="dist_ps", bufs=1, space="PSUM"))

        for b in range(BL):
            mins0 = mins_pool.tile([128, 8], F32, tag="m0")
            mins1 = mins_pool.tile([128, 8], F32, tag="m1")
            mins = [mins0, mins1]
            for ti, w0 in enumerate(TLOADS):
                tt = t_pool.tile([128, TCOLS], XDT)
                nc.sync.dma_start(
                    out=tt,
                    in_=bass.AP(
                        tensor=data.tensor,
                        offset=data.offset + b * FLAT + 3 * w0,
                        ap=[[48, 128], [1, TCOLS]],
                    ),
                )
                dd = d_pool.tile([128, DCOLS], XDT)
                nc.sync.dma_start(
                    out=dd,
                    in_=bass.AP(
                        tensor=d2s.tensor,
                        offset=d2s.offset + b * T + w0,
                        ap=[[16, 128], [1, DCOLS]],
                    ),
                )
                # gpsimd assembles per-q [x(96) | d2(32)] tiles from the
                # compact stride-16 loads (PE matmul dst must start at
                # partition 0, so a single combined transpose per q is the
                # only legal way to build the [128, 128] rhs block).
                xq = xq_pool.tile([128, 16, 128], XDT)
                nc.scalar.copy(
                    xq[:, :, 0:KC],
                    bass.AP(tensor=tt.tensor, offset=tt.offset,
                            ap=[tt.ap[0], [3, 16], [1, KC]]),
                )
                nc.scalar.copy(
                    xq[:, :, KC:128],
                    bass.AP(tensor=dd.tensor, offset=dd.offset,
                            ap=[dd.ap[0], [1, 16], [1, LS]]),
                )
                for half in range(2):  # 2 min-groups per T-load
                    g = ti * 2 + half
                    dist0 = dist_ps.tile([128, 2, 512], F32, tag="d0")
                    dist1 = dist_ps.tile([128, 2, 512], F32, tag="d1")
                    dist = [dist0, dist1]
                    xts_tiles = []
                    for i in range(2):  # 2 rhs tiles per min-group
                        gq = (half * 2 + i) * 4
                        trp = tr_ps.tile([128, 512], XDT)
                        for qq in range(4):
                            q = gq + qq
                            blk = slice(qq * 128, (qq + 1) * 128)
                            nc.tensor.transpose(trp[:, blk], xq[:, q, :], identx)
                        xts = xts_pool.tile([128, 512], XDT)
                        if USE_BF16:
                            nc.vector.tensor_copy(xts, trp)
                        else:
                            nc.scalar.copy(xts, trp)
                        xts_tiles.append(xts)
                    for sc in range(2):  # lhsT-major: one kaug load per 2 MMs
                        for i in range(2):
                            nc.tensor.matmul(
                                dist[sc][:, i, :],
                                kaug[sc],
                                xts_tiles[i],
                                start=True,
                                stop=True,
                            )
                    # reduce this group: min over its windows, per shapelet
                    for sc in range(2):
                        # direct 1x DVE reduce from PSUM; the DVE has a single
                        # PSUM read port, so no dual-operand fusion can beat it
                        nc.vector.tensor_reduce(
                            mins[sc][:, g : g + 1],
                            dist[sc],
                            axis=mybir.AxisListType.XY,
                            op=mybir.AluOpType.min,
                        )
            for sc in range(2):
                res = fin_pool.tile([128, 1], F32, tag="res")
                nc.vector.tensor_reduce(
                    res,
                    mins[sc],
                    axis=mybir.AxisListType.X,
                    op=mybir.AluOpType.min,
                )
                fin = fin_pool.tile([128, 1], F32, tag="fin")
                nc.vector.tensor_scalar(
                    out=fin,
                    in0=res,
                    scalar1=k2c[sc],
                    scalar2=None,
                    op0=mybir.AluOpType.add,
                )
                nc.sync.dma_start(
                    out=out[b, sc * 128 : (sc + 1) * 128].rearrange(
                        "(p o) -> p o", o=1
                    ),
                    in_=fin,
                )
    nc.compile()
    return nc


_PROGRAM = None


def _get_program() -> bass.Bass:
    global _PROGRAM
    if _PROGRAM is None:
        _PROGRAM = build_program()
    return _PROGRAM


def _make_pre(kflat: np.ndarray) -> np.ndarray:
    pre = np.empty((128, 320), dtype=np.float32)
    pre[:, 0:KC] = kflat[0:128]
    pre[:, KC : 2 * KC] = kflat[128:256]
    pre[:, 2 * KC : 320] = np.eye(128, dtype=np.float32)
    return pre


def make_in_maps(data: np.ndarray, kernel: np.ndarray) -> list[dict]:
    assert data.shape == (B, T, C) and kernel.shape == (NS, LS, C)
    flat = np.ascontiguousarray(data, dtype=np.float32).reshape(B, FLAT)
    kflat = np.ascontiguousarray(kernel, dtype=np.float32).reshape(NS, KC)
    if USE_BF16:
        flat = flat.astype(ml_dtypes.bfloat16)
        # round the kernel through bf16 so k2 is consistent with the bf16
        # kaug actually used in the matmul (output = exact distance of the
        # rounded vectors)
        kflat = kflat.astype(ml_dtypes.bfloat16).astype(np.float32)
    pre = _make_pre(kflat)
    maps = [
        {"data": np.ascontiguousarray(flat[i * BL : (i + 1) * BL]), "pre": pre}
        for i in range(NCORES)
    ]
    if USE_BF16:
        preb = np.eye(128, dtype=np.float32).astype(ml_dtypes.bfloat16)
        for m in maps:
            m["preb"] = preb
    return maps


def kernel(data: np.ndarray, kernel: np.ndarray) -> np.ndarray:
    from concourse.bass_utils import run_bass_kernel_spmd

    in_maps = make_in_maps(data, kernel)
    nc = _get_program()
    res = run_bass_kernel_spmd(nc, in_maps, list(range(NCORES)))
    return np.concatenate(
        [res.results[i]["out"] for i in range(NCORES)], axis=0
    ).astype(np.float32)



# revision 21
# speedup vs baseline: 1.2261x; 1.2261x over previous
"""Shapelet distance transform kernel for Trainium2 (8 NeuronCores).

out[b, s] = min_w sum_{l,c} (data[b, w+l, c] - kernel[s, l, c])^2 / LS

Strategy (data-parallel over batch, 4 batches per core, no collectives):
  dist[s, w] = (a2[w] + k2[s] - 2*cross[s, w]) / LS

Per core, per batch:
  - d2[t] = sum_c data[t, c]^2 staged to DRAM scratch (one pass).
  - Stride-16 window loads: T[p, j] = flat[3*(w0 + 16p) + j], shape [128, 141],
    contains the im2col rows of 2048 windows as column slices T[:, 3q:3q+96]
    (window w0 + 16p + q lives in partition p, slice q). One 73 KB DMA with
    564-byte contiguous lines covers 2048 windows - the min over windows is
    order-agnostic so the odd window order is free.
  - D[p, j] = d2[w0 + 16p + j], shape [128, 47]; slice D[:, q:q+32] holds the
    32 sliding-norm taps of the same windows.
  - ACT assembles per-q [x(96) | d2(32)] column blocks (PE matmul dst
    must start at partition 0, so one combined transpose per q is used);
    ACT copies 4 transposed blocks to SBUF as the matmul rhs [128, 512].
  - One float32r matmul per (s-chunk, rhs tile): lhsT = Kaug[128, 128] with
    rows [0:96] = -2*kernel^T/LS and rows [96:128] = 1/LS (turning the taps
    into a2[w]/LS), so PSUM = (a2 - 2*cross)/LS directly.
  - min over w: DVE tensor_reduce(min) straight from PSUM (the DVE's single
    PSUM read port makes 1 elem/lane/cycle the hard floor; offload paths all
    measured slower).
  - + k2[s]/LS, then DMA out.
"""

import sys

for _p in ("/opt/trn_rl_repo",):
    if _p not in sys.path:
        sys.path.insert(0, _p)

from contextlib import ExitStack

import ml_dtypes
import numpy as np

import concourse.bacc as bacc
import concourse.bass as bass
import concourse.tile as tile
from concourse import mybir

F32 = mybir.dt.float32
F32R = mybir.dt.float32r
BF16 = mybir.dt.bfloat16

# X-side dtype: bfloat16 halves PE stream/weight-load time at ~1e-3 rel err;
# float32r gives ~1.7e-4 rel err at ~2x the PE cost.
USE_BF16 = False
XDT = BF16 if USE_BF16 else F32R

B, T, C = 32, 8192, 3
NS, LS = 256, 32
W = T - LS + 1  # 8161 valid windows
NCORES = 8
BL = B // NCORES  # 4 batches per core
FLAT = T * C  # 24576
KC = LS * C  # 96 im2col columns from data
SCALE = 1.0 / LS

# Each T-load covers 2048 windows: w = w0 + 16p + q, p in [0,128), q in [0,16).
# Final load overlaps so that [0, W) is covered exactly in-bounds.
TLOADS = [0, 2048, 4096, W - 2048]
TCOLS = 3 * 15 + KC  # 141
DCOLS = 15 + LS  # 47
# Per T-load: 4 rhs tiles (q-groups of 4), paired into 2 PSUM min-groups.
# Per (batch, s-chunk): 8 min-groups. Groups routed via ACT copy + 2x DVE min:
ROUTED = set()  # HW: routed min costs MORE DVE time than direct PSUM reduce


def build_program() -> bass.Bass:
    nc = bacc.Bacc("TRN2", target_bir_lowering=False, debug=False)
    data = nc.dram_tensor("data", [BL, FLAT], XDT, kind="ExternalInput").ap()
    # pre[:, 0:96]=kern rows 0:128, [:, 96:192]=kern rows 128:256,
    # [:, 192:320]=identity(128); preb = bf16 identity (streams 2x faster)
    pre = nc.dram_tensor("pre", [128, 320], F32R, kind="ExternalInput").ap()
    preb = (
        nc.dram_tensor("preb", [128, 128], BF16, kind="ExternalInput").ap()
        if USE_BF16
        else None
    )
    out = nc.dram_tensor("out", [BL, NS], F32, kind="ExternalOutput").ap()
    d2s = nc.dram_tensor("d2s", [BL, T], XDT).ap()

    with tile.TileContext(nc) as tc, ExitStack() as ctx:
        consts = ctx.enter_context(tc.tile_pool(name="consts", bufs=1))
        pre_sb = consts.tile([128, 320], F32R)
        nc.sync.dma_start(out=pre_sb, in_=pre)
        ident = pre_sb[:, 192:320]
        if USE_BF16:
            identb = consts.tile([128, 128], BF16)
            nc.sync.dma_start(out=identb, in_=preb)
            identx = identb
        else:
            identx = ident
        kaug0 = consts.tile([128, 128], XDT)
        kaug1 = consts.tile([128, 128], XDT)
        kaug = [kaug0, kaug1]
        k2c0 = consts.tile([128, 1], F32)
        k2c1 = consts.tile([128, 1], F32)
        k2c = [k2c0, k2c1]

        # Phase-A pools stay alive for the whole kernel: releasing them would
        # add PSUM/SBUF reuse dependencies (and walrus-visible waits) onto the
        # first main-loop instructions. prep_ps keeps one dedicated PSUM bank
        # that later doubles as the dummy-transpose scratch.
        prep = ctx.enter_context(tc.tile_pool(name="prep", bufs=2))
        prep_ps = ctx.enter_context(tc.tile_pool(name="prep_ps", bufs=1, space="PSUM"))

        # ---- Phase A: stage d2, build Kaug and k2/LS columns (Tile
        # schedules by dependencies; program order here is not critical). ----
        for b in range(BL):
            dt_ = prep.tile([128, 192], XDT, tag="dt")
            nc.scalar.dma_start(out=dt_, in_=data[b].rearrange("(p f) -> p f", p=128))
            sq = prep.tile([128, 192], F32, tag="sq")
            nc.scalar.square(sq, dt_)
            d2t = prep.tile([128, 64], XDT, tag="d2t")
            # float32r is 4-byte fp32 storage; the accumulate is fp32.
            with nc.allow_low_precision(reason="f32r is fp32-width"):
                nc.vector.tensor_reduce(
                    d2t,
                    sq.rearrange("p (t c) -> p t c", c=3),
                    axis=mybir.AxisListType.X,
                    op=mybir.AluOpType.add,
                )
            nc.scalar.dma_start(out=d2s[b].rearrange("(p f) -> p f", p=128), in_=d2t)

        for sc in range(2):
            ksb = pre_sb[:, sc * KC : (sc + 1) * KC]
            ksq = prep.tile([128, KC], F32, tag="ksq")
            nc.vector.tensor_mul(ksq, ksb, ksb)
            k2raw = prep.tile([128, 1], F32, tag="k2raw")
            nc.vector.tensor_reduce(
                k2raw, ksq, axis=mybir.AxisListType.X, op=mybir.AluOpType.add
            )
            nc.vector.tensor_scalar_mul(k2c[sc], k2raw, SCALE)
            ktp = prep_ps.tile([KC, 128], F32R, tag="scratch")
            nc.tensor.transpose(ktp, ksb, ident)
            # rows 0:96 = -2/LS * K^T ; rows 96:128 = 1/LS (a2 taps)
            nc.scalar.activation(
                kaug[sc][0:KC, :],
                ktp,
                mybir.ActivationFunctionType.Copy,
                scale=-2.0 * SCALE,
            )
            # (memset can't target float32r; write the constant via DVE)
            nc.vector.tensor_scalar(
                out=kaug[sc][KC:128, :],
                in0=pre_sb[0:LS, 0:128],
                scalar1=0.0,
                scalar2=SCALE,
                op0=mybir.AluOpType.mult,
                op1=mybir.AluOpType.add,
            )

        # ---- Phase B: main loop. ----
        t_pool = ctx.enter_context(tc.tile_pool(name="tp", bufs=3))
        xq_pool = ctx.enter_context(tc.tile_pool(name="xq", bufs=4))
        d_pool = ctx.enter_context(tc.tile_pool(name="dp", bufs=3))
        xts_pool = ctx.enter_context(tc.tile_pool(name="xts", bufs=6))
        mins_pool = ctx.enter_context(tc.tile_pool(name="mins", bufs=4))
        fin_pool = ctx.enter_context(tc.tile_pool(name="fin", bufs=8))
        tr_ps = ctx.enter_context(tc.tile_pool(name="tr_ps", bufs=3, space="PSUM"))
        dist_ps = ctx.enter_context(tc.tile_pool(name="dist_ps", bufs=1, space="PSUM"))

        for b in range(BL):
            mins0 = mins_pool.tile([128, 8], F32, tag="m0")
            mins1 = mins_pool.tile([128, 8], F32, tag="m1")
            mins = [mins0, mins1]
            for ti, w0 in enumerate(TLOADS):
                tt = t_pool.tile([128, TCOLS], XDT)
                nc.sync.dma_start(
                    out=tt,
                    in_=bass.AP(
                        tensor=data.tensor,
                        offset=data.offset + b * FLAT + 3 * w0,
                        ap=[[48, 128], [1, TCOLS]],
                    ),
                )
                dd = d_pool.tile([128, DCOLS], XDT)
                nc.sync.dma_start(
                    out=dd,
                    in_=bass.AP(
                        tensor=d2s.tensor,
                        offset=d2s.offset + b * T + w0,
                        ap=[[16, 128], [1, DCOLS]],
                    ),
                )
                # gpsimd assembles per-q [x(96) | d2(32)] tiles from the
                # compact stride-16 loads (PE matmul dst must start at
                # partition 0, so a single combined transpose per q is the
                # only legal way to build the [128, 128] rhs block).
                xq = xq_pool.tile([128, 16, 128], XDT)
                nc.scalar.copy(
                    xq[:, :, 0:KC],
                    bass.AP(tensor=tt.tensor, offset=tt.offset,
                            ap=[tt.ap[0], [3, 16], [1, KC]]),
                )
                nc.scalar.copy(
                    xq[:, :, KC:128],
                    bass.AP(tensor=dd.tensor, offset=dd.offset,
                            ap=[dd.ap[0], [1, 16], [1, LS]]),
                )
                for half in range(2):  # 2 min-groups per T-load
                    g = ti * 2 + half
                    dist0 = dist_ps.tile([128, 2, 512], F32, tag="d0")
                    dist1 = dist_ps.tile([128, 2, 512], F32, tag="d1")
                    dist = [dist0, dist1]
                    xts_tiles = []
                    for i in range(2):  # 2 rhs tiles per min-group
                        gq = (half * 2 + i) * 4
                        trp = tr_ps.tile([128, 512], XDT)
                        for qq in range(4):
                            q = gq + qq
                            blk = slice(qq * 128, (qq + 1) * 128)
                            nc.tensor.transpose(trp[:, blk], xq[:, q, :], identx)
                        xts = xts_pool.tile([128, 512], XDT)
                        if USE_BF16:
                            nc.vector.tensor_copy(xts, trp)
                        else:
                            nc.scalar.copy(xts, trp)
                        xts_tiles.append(xts)
                    for sc in range(2):  # lhsT-major: one kaug load per 2 MMs
                        for i in range(2):
                            nc.tensor.matmul(
                                dist[sc][:, i, :],
                                kaug[sc],
                                xts_tiles[i],
                                start=True,
                                stop=True,
                            )
                    # reduce this group: min over its windows, per shapelet
                    for sc in range(2):
                        # direct 1x DVE reduce from PSUM; the DVE has a single
                        # PSUM read port, so no dual-operand fusion can beat it
                        nc.vector.tensor_reduce(
                            mins[sc][:, g : g + 1],
                            dist[sc],
                            axis=mybir.AxisListType.XY,
                            op=mybir.AluOpType.min,
                        )
            for sc in range(2):
                res = fin_pool.tile([128, 1], F32, tag="res")
                nc.vector.tensor_reduce(
                    res,
                    mins[sc],
                    axis=mybir.AxisListType.X,
                    op=mybir.AluOpType.min,
                )
                fin = fin_pool.tile([128, 1], F32, tag="fin")
                nc.vector.tensor_scalar(
                    out=fin,
                    in0=res,
                    scalar1=k2c[sc],
                    scalar2=None,
                    op0=mybir.AluOpType.add,
                )
                nc.sync.dma_start(
                    out=out[b, sc * 128 : (sc + 1) * 128].rearrange(
                        "(p o) -> p o", o=1
                    ),
                    in_=fin,
                )
    nc.compile()
    return nc


_PROGRAM = None


def _get_program() -> bass.Bass:
    global _PROGRAM
    if _PROGRAM is None:
        _PROGRAM = build_program()
    return _PROGRAM


def _make_pre(kflat: np.ndarray) -> np.ndarray:
    pre = np.empty((128, 320), dtype=np.float32)
    pre[:, 0:KC] = kflat[0:128]
    pre[:, KC : 2 * KC] = kflat[128:256]
    pre[:, 2 * KC : 320] = np.eye(128, dtype=np.float32)
    return pre


def make_in_maps(data: np.ndarray, kernel: np.ndarray) -> list[dict]:
    assert data.shape == (B, T, C) and kernel.shape == (NS, LS, C)
    flat = np.ascontiguousarray(data, dtype=np.float32).reshape(B, FLAT)
    kflat = np.ascontiguousarray(kernel, dtype=np.float32).reshape(NS, KC)
    if USE_BF16:
        flat = flat.astype(ml_dtypes.bfloat16)
        # round the kernel through bf16 so k2 is consistent with the bf16
        # kaug actually used in the matmul (output = exact distance of the
        # rounded vectors)
        kflat = kflat.astype(ml_dtypes.bfloat16).astype(np.float32)
    pre = _make_pre(kflat)
    maps = [
        {"data": np.ascontiguousarray(flat[i * BL : (i + 1) * BL]), "pre": pre}
        for i in range(NCORES)
    ]
    if USE_BF16:
        preb = np.eye(128, dtype=np.float32).astype(ml_dtypes.bfloat16)
        for m in maps:
            m["preb"] = preb
    return maps


def kernel(data: np.ndarray, kernel: np.ndarray) -> np.ndarray:
    from concourse.bass_utils import run_bass_kernel_spmd

    in_maps = make_in_maps(data, kernel)
    nc = _get_program()
    res = run_bass_kernel_spmd(nc, in_maps, list(range(NCORES)))
    return np.concatenate(
        [res.results[i]["out"] for i in range(NCORES)], axis=0
    ).astype(np.float32)

